# revision 43
# baseline (speedup 1.0000x reference)
"""Trainium2 Bass kernel for nn_Mhsa_47802986004933.

Model (per batch b of 2):
  BN(train-stats)+ReLU -> 1x1 conv qkv (raw .view reinterpret) ->
  4-head attention on heads 0-3  +  conv-mixing (3x1 / 1x3) on heads 4-7 ->
  concat -> kernel-2 avg pool.

Sharding: 8 cores = (batch b in {0,1}) x (h in {0..3}).
  Core c = 4b + h:
    - full 4096x4096 attention for head h of batch b  -> out[b, :, 32h:32h+32]
    - conv y-quarter [16h, 16h+16)                    -> out[b, n%16 in [4h,4h+4), 128:256]
  Communication-free SPMD: BN stats recomputed on every core from the full x.

Key structural identity: with O = W @ xn [1536, 4096] per batch and
U = O.reshape(12288, 512) (u = 8o+g), token n has q = U[3n], k = U[3n+1],
v = U[3n+2].  Attention head h uses U columns [64h, 64h+64); the conv branch
uses columns [256, 512) with image layout q2[i, y, x] =
U[3*(64*(i%64)+y), 256 + 64*(i//64) + x].

All matmuls run as float32r (f32 storage, 1 cycle/row on the PE at N>=256).
"""
import os
import sys
import numpy as np
import ml_dtypes

sys.path.insert(0, "/opt/trn_rl_repo")

import concourse.bass as bass
import concourse.bacc as bacc
import concourse.mybir as mybir
import concourse.tile as tile
from concourse import bass_utils

B, N, DIM, S = 2, 4096, 256, 64
H, DH, INNER = 8, 64, 512
EPS = 1e-5
FP = mybir.dt.float32
FR = mybir.dt.float32r
BF = mybir.dt.bfloat16
AF = mybir.ActivationFunctionType
OP = mybir.AluOpType

# scores exp groups per 512-query chunk: 16 uniform glen-2 groups over
# 3 rotating PSUM pools (pipeline depth 3).  10 groups -> exact exp on
# ACT; 6 groups -> fastexp bit-trick on DVE.
GROUPS = [2] * 16
ACT_GROUPS = {0, 2, 3, 5, 6, 8, 11, 12, 14}
LOG2E = 1.4426950408889634
FE_A = 16.0 * LOG2E              # 128*log2e*0.125 applied to raw scores
FE_B = 16248.0 + 12582912.0      # bias-8 + 1.5*2^23 round-to-int magic


def _r(ap):
    return ap.bitcast(FR)


def build_device_program():
    nc = bacc.Bacc(
        "TRN2", target_bir_lowering=False, debug=False, enable_asserts=True,
        num_devices=8,
    )
    xcb_d = nc.dram_tensor("xcb", [256, 2560], BF, kind="ExternalInput").ap()
    xrb_d = nc.dram_tensor("xrb", [256, 5632], BF, kind="ExternalInput").ap()
    wq_d = nc.dram_tensor("wq", [256, 1536], BF, kind="ExternalInput").ap()
    wcg_d = nc.dram_tensor("wcg", [256, 3072], BF, kind="ExternalInput").ap()
    wch_d = nc.dram_tensor("wch", [256, 128], BF, kind="ExternalInput").ap()
    w1s_d = nc.dram_tensor("w1s", [256, 768], BF, kind="ExternalInput").ap()
    w2s_d = nc.dram_tensor("w2s", [256, 768], BF, kind="ExternalInput").ap()
    gb_d = nc.dram_tensor("gb", [256, 2], FP, kind="ExternalInput").ap()
    idn_d = nc.dram_tensor("idn", [128, 128], FP, kind="ExternalInput").ap()
    out_a = nc.dram_tensor("out_a", [4096, 32], FP, kind="ExternalOutput").ap()
    out_c = nc.dram_tensor("out_c", [1024, 128], FP, kind="ExternalOutput").ap()

    with tile.TileContext(nc) as tc:
        _emit(tc, nc, xcb_d, xrb_d, wq_d, wcg_d, wch_d, w1s_d, w2s_d,
              gb_d, idn_d, out_a, out_c)
    nc.compile()
    return nc


def _emit(tc, nc, xcb_d, xrb_d, wq_d, wcg_d, wch_d, w1s_d, w2s_d, gb_d,
          idn_d, out_a, out_c):
    from contextlib import ExitStack
    ctx = ExitStack()
    with ctx:
        cp = ctx.enter_context(tc.tile_pool(name="const", bufs=1))
        sctx = ExitStack()
        sp = sctx.enter_context(tc.tile_pool(name="scratch", bufs=1))
        xp = sctx.enter_context(tc.tile_pool(name="xload", bufs=2))
        pa_ = ctx.enter_context(tc.tile_pool(name="ps_a", bufs=1, space="PSUM"))
        pb_ = ctx.enter_context(tc.tile_pool(name="ps_b", bufs=1, space="PSUM"))
        pc_ = ctx.enter_context(tc.tile_pool(name="ps_c", bufs=1, space="PSUM"))
        po = ctx.enter_context(tc.tile_pool(name="ps_o", bufs=2, space="PSUM"))

        # front-phase matmul staging: rotate over po's two banks plus the
        # three (not-yet-started) attention score pools -> 5 slots, so the
        # PSUM->SBUF copy latency never gates the next front matmul.
        fr_cnt = [0]

        def ftile(name):
            i = fr_cnt[0] % 5
            fr_cnt[0] += 1
            if i in (0, 2):
                return po.tile([128, 512], FP, tag="o", name=name)
            pool = {1: None, 3: None}
            p = (pa_, pb_, pc_)[(1, 0, 3, 0, 4)[i] - 1] if False else                 {1: pa_, 3: pb_, 4: pc_}[i]
            return p.tile([128, 1024], FP, tag="s", name=name)[:, 0:512]

        # ---------------- persistent SBUF ----------------
        wq = [cp.tile([128, 1536], BF, tag="wq0", name="wq0"),
              cp.tile([128, 1536], BF, tag="wq1", name="wq1")]
        w1s = [cp.tile([128, 768], BF, tag="w1s0", name="w1s0"),
               cp.tile([128, 768], BF, tag="w1s1", name="w1s1")]
        w2s = [cp.tile([128, 768], BF, tag="w2s0", name="w2s0"),
               cp.tile([128, 768], BF, tag="w2s1", name="w2s1")]
        gb = [cp.tile([128, 2], FP, tag="gb0", name="gb0"),
              cp.tile([128, 2], FP, tag="gb1", name="gb1")]
        idn = cp.tile([128, 128], FR, tag="idn", name="idn")
        ut = cp.tile([64, 12288], FR, tag="ut", name="ut")
        vsb = cp.tile([128, 2080], BF, tag="vsb", name="vsb")
        sq2 = sp.tile([128, 2048], BF, tag="sq2", name="sq2")  # slim q (j=0)
        sk2 = sp.tile([128, 2048], BF, tag="sk2", name="sk2")
        sv2 = sp.tile([128, 2048], BF, tag="sv2", name="sv2")
        qhalo = [sp.tile([64, 256], BF, tag="qhalo0", name="qhalo0"),
                 sp.tile([64, 256], BF, tag="qhalo1", name="qhalo1")]
        q2q = [cp.tile([128, 1152], BF, tag="q2q0", name="q2q0"),
               cp.tile([128, 1152], BF, tag="q2q1", name="q2q1")]
        k2q = [cp.tile([128, 1024], BF, tag="k2q0", name="k2q0"),
               cp.tile([128, 1024], BF, tag="k2q1", name="k2q1")]
        v2q = [cp.tile([128, 1024], BF, tag="v2q0", name="v2q0"),
               cp.tile([128, 1024], BF, tag="v2q1", name="v2q1")]

        # transient loads (scratch pool, released before k2l/k2r + work pools)
        wch = [sp.tile([128, 128], BF, tag="wch0", name="wch0"),
               sp.tile([128, 128], BF, tag="wch1", name="wch1")]
        xcld = [cp.tile([128, 2560], BF, tag="xc0", name="xc0"),
                cp.tile([128, 2560], BF, tag="xc1", name="xc1")]
        xr = [sp.tile([128, 5632], BF, tag="xr0", name="xr0"),
              sp.tile([128, 5632], BF, tag="xr1", name="xr1")]
        bnst = [sp.tile([128, 96], FP, tag="bnst0", name="bnst0"),
                sp.tile([128, 96], FP, tag="bnst1", name="bnst1")]

        dma = nc.sync.dma_start
        vec = nc.vector
        act = nc.scalar

        zconst = cp.tile([128, 1], FP, tag="zconst", name="zconst")
        vec.memset(zconst, 0.0)
        nc.const_aps.aps[(FP, 0.0)] = zconst
        epst = cp.tile([128, 1], FP, tag="epst", name="epst")
        vec.memset(epst, EPS)
        onec = cp.tile([128, 1], FP, tag="onec", name="onec")
        vec.memset(onec, 1.0)

        # attention work tiles (declared before scratch release; manual 2x alt)
        psbs = [cp.tile([128, 1024], BF, tag=f"psb{i}", name=f"psb{i}")
                for i in range(3)]
        fscs = [cp.tile([128, 1024], FP, tag=f"fsc{i}", name=f"fsc{i}")
                for i in range(3)]
        resbs = [cp.tile([128, 128], FP, tag="resbA", name="resbA"),
                 cp.tile([128, 128], FP, tag="resbB", name="resbB")]
        recs = [cp.tile([128, 1], FP, tag="recA", name="recA"),
                cp.tile([128, 1], FP, tag="recB", name="recB")]
        pav2s = [cp.tile([128, 64], FP, tag="pav2A", name="pav2A"),
                 cp.tile([128, 64], FP, tag="pav2B", name="pav2B")]

        # ---------------- BN stats over full x (critical path: DMA first) ----
        # split: DVE bn_stats on xcld + xr[0:3072]; ACT copy/square-accum on
        # xr[3072:4608]; Pool square+reduce on xr[4608:5632]; tiny merge.
        for hf in range(2):
            dma(out=xcld[hf], in_=xcb_d[128 * hf:128 * hf + 128, :])
        for hf in range(2):
            for lo, hi in ((0, 3072), (3072, 4608), (4608, 5632)):
                dma(out=xr[hf][:, lo:hi],
                    in_=xrb_d[128 * hf:128 * hf + 128, lo:hi])

        # PE warmup: junk matmuls on loaded x to ramp the clock before the
        # projection phase (outputs never read)
        for wi in range(30):
            jt = ftile(f"warm{wi}")
            nc.tensor.matmul(jt[:, 0:512], (xr[0][:, 0:128]),
                             (xr[0][:, 512:1024]), start=True, stop=True,
                             skip_group_check=True)

        gps = nc.gpsimd
        mvs = []
        sjnk = sp.tile([128, 2560], BF, tag="sjnk", name="sjnk")
        for hf in range(2):
            # ACT leg: S1a/S2a over xr[3072:4608]
            s1a = sp.tile([128, 1], FP, tag="s1a", bufs=2, name=f"s1a{hf}")
            s2a = sp.tile([128, 1], FP, tag="s2a", bufs=2, name=f"s2a{hf}")
            act.activation(sjnk, xr[hf][:, 3072:5632], AF.Copy, accum_out=s1a)
            act.activation(sjnk, xr[hf][:, 3072:5632], AF.Square,
                           accum_out=s2a)
            # DVE leg: bn_stats over xcld (5 chunks) + xr[0:3072] (6 chunks)
            for ck in range(5):
                vec.bn_stats(out=bnst[hf][:, 6 * ck:6 * ck + 6],
                             in_=xcld[hf][:, 512 * ck:512 * ck + 512])
            for ck in range(6):
                vec.bn_stats(out=bnst[hf][:, 6 * (5 + ck):6 * (5 + ck) + 6],
                             in_=xr[hf][:, 512 * ck:512 * ck + 512])
            mvd = sp.tile([128, 2], FP, tag="mvd", bufs=2, name=f"mvd{hf}")
            vec.bn_aggr(out=mvd, in_=bnst[hf][:, 0:66].rearrange(
                "p (k s) -> p k s", s=6))
            # merge: N_d=5632 (DVE), N_r=2560 (ACT+Pool), N=8192
            mv = sp.tile([128, 2], FP, tag="mv", bufs=2, name=f"mv{hf}")
            t1 = sp.tile([128, 4], FP, tag="mt", bufs=2, name=f"mt{hf}")
            # t1[0] = S1 ; t1[1] = S2
            gps.tensor_copy(t1[:, 0:1], s1a)
            gps.tensor_copy(t1[:, 1:2], s2a)
            # t1[2] = mean_d^2 ; t1[3] = (var_d + mean_d^2) * N_d  (= E2_d*N_d)
            gps.tensor_tensor(t1[:, 2:3], mvd[:, 0:1], mvd[:, 0:1], OP.mult)
            gps.tensor_add(t1[:, 3:4], mvd[:, 1:2], t1[:, 2:3])
            gps.tensor_scalar(t1[:, 3:4], t1[:, 3:4], 5632.0, None, OP.mult)
            # mv[0] = mean = (N_d*mean_d + S1)/N
            gps.tensor_scalar(mv[:, 0:1], mvd[:, 0:1], 5632.0, None, OP.mult)
            gps.tensor_add(mv[:, 0:1], mv[:, 0:1], t1[:, 0:1])
            gps.tensor_scalar(mv[:, 0:1], mv[:, 0:1], 1.0 / 8192.0, None,
                              OP.mult)
            # mv[1] = var = (E2_d*N_d + S2)/N - mean^2
            gps.tensor_add(t1[:, 3:4], t1[:, 3:4], t1[:, 1:2])
            gps.tensor_scalar(t1[:, 3:4], t1[:, 3:4], 1.0 / 8192.0, None,
                              OP.mult)
            gps.tensor_tensor(t1[:, 2:3], mv[:, 0:1], mv[:, 0:1], OP.mult)
            gps.tensor_sub(mv[:, 1:2], t1[:, 3:4], t1[:, 2:3])
            mvs.append(mv)

        # ---------------- loads (wq next on the critical path) ----------
        for hf in range(2):
            dma(out=gb[hf], in_=gb_d[128 * hf:128 * hf + 128, :])
        for hf in range(2):
            dma(out=wq[hf], in_=wq_d[128 * hf:128 * hf + 128, :])
        for hf in range(2):
            dma(out=w1s[hf], in_=w1s_d[128 * hf:128 * hf + 128, :])
            dma(out=w2s[hf], in_=w2s_d[128 * hf:128 * hf + 128, :])
            dma(out=wch[hf], in_=wch_d[128 * hf:128 * hf + 128, :])
        dma(out=idn, in_=idn_d.bitcast(FR))

        # affine: a = gamma*rsqrt(var+eps); bb = beta - mean*a
        aff = []
        for hf in range(2):
            sqv = sp.tile([128, 1], FP, tag="sqv", bufs=2, name=f"sqv{hf}")
            act.activation(sqv, mvs[hf][:, 1:2], AF.Sqrt, bias=epst)
            rsv = sp.tile([128, 1], FP, tag="rsv", bufs=2, name=f"rsv{hf}")
            vec.reciprocal(rsv, sqv)
            a_ = sp.tile([128, 1], FP, tag="a_", bufs=2, name=f"a{hf}")
            vec.tensor_tensor(a_, rsv, gb[hf][:, 0:1], OP.mult)
            tmp = sp.tile([128, 1], FP, tag="tmp", bufs=2, name=f"tmp{hf}")
            vec.tensor_tensor(tmp, mvs[hf][:, 0:1], a_, OP.mult)
            bb = sp.tile([128, 1], FP, tag="bb", bufs=2, name=f"bb{hf}")
            vec.tensor_tensor(bb, gb[hf][:, 1:2], tmp, OP.subtract)
            aff.append((a_, bb))

        # xn = relu(a*x + b)   (ACT is idle this early; in-place on the load)
        xn = xcld
        for hf in range(2):
            a_, bb = aff[hf]
            act.activation(xn[hf], xn[hf], AF.Relu, bias=bb, scale=a_)

        # ---------------- head projection -> ut  (+ V tiles) ----------------
        # ut[:, 8o+g] = OUT_g[:, o];  OUT_g = xn_head_g^T @ WT.  oc-major so
        # V-tile transposes can chase completed ut column ranges.
        vec.tensor_copy(vsb[:, 64::65], onec.to_broadcast((128, 32)))
        vt = 0
        for oc in range(3):
            for g in range(8):
                ps = ftile(f"pr{g}_{oc}")[0:64, 0:512]
                nc.tensor.matmul(ps, (xn[0][:, 64 * g:64 * g + 64]),
                                 (wq[0][:, 512 * oc:512 * oc + 512]),
                                 start=True, stop=False)
                nc.tensor.matmul(ps, (xn[1][:, 64 * g:64 * g + 64]),
                                 (wq[1][:, 512 * oc:512 * oc + 512]),
                                 start=False, stop=True)
                dst = ut[:, 4096 * oc + g: 4096 * oc + g + 4089: 8]
                if (g + oc) % 2 == 0:
                    vec.tensor_copy(dst, ps)
                else:
                    act.activation(dst, ps, AF.Copy)
            while vt < 32 and 384 * vt + 384 <= 4096 * (oc + 1):
                pv = ftile(f"vt{vt}").bitcast(FR)[:, 0:64]
                nc.tensor.transpose(
                    pv, ut[:, 3 * 128 * vt + 2: 3 * 128 * vt + 384: 3],
                    idn[0:64, 0:64])
                if vt % 2 == 0:
                    vec.tensor_copy(vsb[:, 65 * vt:65 * vt + 64], pv)
                else:
                    act.activation(vsb[:, 65 * vt:65 * vt + 64], pv, AF.Copy)
                vt += 1

        ABL = ""  # ablation switch used only during development
        # ---------------- conv-input slim projection ----------------
        # sX[64*mr+ilo, 256*rho+f] = U[3*(8*(8*ilo+2h+mr)+rho)+j, 256+f]
        for j, dst in (() if ABL == "noconv" else ((0, sq2), (1, sk2), (2, sv2))):
            wcgj = [sp.tile([128, 1024], BF, tag="wcgj0", name=f"wcgj0_{j}"),
                    sp.tile([128, 1024], BF, tag="wcgj1", name=f"wcgj1_{j}")]
            for hf in range(2):
                dma(out=wcgj[hf], in_=wcg_d[
                    128 * hf:128 * hf + 128, 1024 * j:1024 * j + 1024])
            for rho in range(8):
                g = (3 * rho + j) % 8
                ps = ftile(f"pc{j}_{rho}")[:, 0:256]
                nc.tensor.matmul(ps, (wcgj[0][:, 128 * rho:128 * rho + 128]),
                                 (xn[0][:, 512 + 256 * g:512 + 256 * g + 256]),
                                 start=True, stop=False)
                nc.tensor.matmul(ps, (wcgj[1][:, 128 * rho:128 * rho + 128]),
                                 (xn[1][:, 512 + 256 * g:512 + 256 * g + 256]),
                                 start=False, stop=True)
                if rho % 2 == 0:
                    vec.tensor_copy(dst[:, 256 * rho:256 * rho + 256], ps)
                else:
                    act.activation(dst[:, 256 * rho:256 * rho + 256], ps,
                                   AF.Copy)

        # halo rows (j=0): lo rho=7 g=5 ; hi rho=0 g=0  (separate 64-part tiles)
        for e, wcol, gg in (() if ABL == "noconv" else ((0, 0, 5), (1, 64, 0))):
            ph = ftile(f"phalo{e}")[0:64, 0:256]
            nc.tensor.matmul(ph, (wch[0][:, wcol:wcol + 64]),
                             (xn[0][:, 512 + 256 * gg:512 + 256 * gg + 256]),
                             start=True, stop=False)
            nc.tensor.matmul(ph, (wch[1][:, wcol:wcol + 64]),
                             (xn[1][:, 512 + 256 * gg:512 + 256 * gg + 256]),
                             start=False, stop=True)
            vec.tensor_copy(qhalo[e], ph)

        # ------------- permutes into conv-image layout (Pool copies) --------
        # dst (64*hh+ilo, 64*yi+x) <- src (64*ya+ilo, 256*yb + 64*(2ci+hh) + x)
        # Engine copies (partition-offset moves) beat DMAs here: no HWDGE
        # descriptor-gen serialization, and Pool is idle in this phase.
        for ci in (() if ABL == "noconv" else range(2)):
            for hh in range(2):
                for srct, dstt, off in ((sq2, q2q, 64), (sk2, k2q, 0), (sv2, v2q, 0)):
                    for ya in range(2):
                        src = srct[64 * ya:64 * ya + 64, :].rearrange(
                            "i (r h x) -> h i r x", r=8, h=4, x=64)[2 * ci + hh]
                        dst = dstt[ci][64 * hh:64 * hh + 64,
                                       off + 512 * ya:off + 512 * ya + 512
                                       ].rearrange("i (r x) -> i r x", x=64)
                        nc.gpsimd.tensor_copy(dst, src)
                for e, dlo, dhi in ((0, 0, 64), (1, 1088, 1152)):
                    src = qhalo[e].rearrange("i (h x) -> h i x", h=4)[2 * ci + hh]
                    nc.gpsimd.tensor_copy(
                        q2q[ci][64 * hh:64 * hh + 64, dlo:dhi], src)

        # release scratch pools; allocate late pools in the freed space
        sctx.close()
        kp = ctx.enter_context(tc.tile_pool(name="late", bufs=1))
        wp = ctx.enter_context(tc.tile_pool(name="work", bufs=2))
        k2l = [kp.tile([128, 1024], BF, tag="k2l0", name="k2l0"),
               kp.tile([128, 1024], BF, tag="k2l1", name="k2l1")]
        k2r = [kp.tile([128, 1024], BF, tag="k2r0", name="k2r0"),
               kp.tile([128, 1024], BF, tag="k2r1", name="k2r1")]

        # k2 shifted-by-x copies with zeroed block edges (SBUF-only -> Pool)
        for ci in (() if ABL == "noconv" else range(2)):
            kv = k2q[ci].rearrange("p (y x) -> p y x", x=64)
            gps.tensor_copy(k2l[ci][:, 63::64], zconst.to_broadcast((128, 16)))
            lv = k2l[ci].rearrange("p (y x) -> p y x", x=64)
            gps.tensor_copy(lv[:, :, 0:63], kv[:, :, 1:64])
            gps.tensor_copy(k2r[ci][:, 0::64], zconst.to_broadcast((128, 16)))
            rv = k2r[ci].rearrange("p (y x) -> p y x", x=64)
            gps.tensor_copy(rv[:, :, 1:64], kv[:, :, 0:63])

        # ---------------- conv matmuls + pair-avg + store ----------------
        v2p = [kp.tile([128, 512], FP, tag="v2p0", name="v2p0"),
               kp.tile([128, 512], FP, tag="v2p1", name="v2p1")]
        pavb = [kp.tile([128, 512], FP, tag="pavb0", name="pavb0"),
                kp.tile([128, 512], FP, tag="pavb1", name="pavb1")]
        for oc in (() if ABL == "noconv" else range(2)):
            vv = v2q[oc].rearrange("p (e two) -> p e two", two=2)
            gps.tensor_add(v2p[oc], vv[:, :, 0], vv[:, :, 1])

        def emit_conv_group(oc, ch, ps):
            k = 0
            for dy in range(3):
                for hf in range(2):
                    nc.tensor.matmul(
                        ps, (w1s[hf][:, 256 * dy + 128 * oc:256 * dy + 128 * oc + 128]),
                        (q2q[hf][:, 512 * ch + 64 * dy:512 * ch + 64 * dy + 512]),
                        start=(k == 0), stop=False, skip_group_check=True)
                    k += 1
            for dx, srcb in ((0, k2r), (1, k2q), (2, k2l)):
                for hf in range(2):
                    nc.tensor.matmul(
                        ps, (w2s[hf][:, 256 * dx + 128 * oc:256 * dx + 128 * oc + 128]),
                        (srcb[hf][:, 512 * ch:512 * ch + 512]),
                        start=False, stop=(k == 11), skip_group_check=True)
                    k += 1
            cop = wp.tile([128, 512], FP, tag="cop", name=f"cop{oc}{ch}")
            if ch % 2 == 0:
                vec.tensor_copy(cop, ps)
            else:
                act.activation(cop, ps, AF.Copy)
            pav = pavb[oc][:, 256 * ch:256 * ch + 256]
            csv = cop.rearrange("p (e two) -> p e two", two=2)
            gps.tensor_add(pav, csv[:, :, 0], csv[:, :, 1])
            gps.tensor_add(pav, pav, v2p[oc][:, 256 * ch:256 * ch + 256])
            if ch == 1:
                dma(out=out_c.rearrange("(o w) e -> o w e", w=4)[
                        128 * oc:128 * oc + 128, :, :],
                    in_=pavb[oc].rearrange("p (w e) -> p w e", w=4))

        # ---------------- attention ----------------
        # scores keys-major -> exp -> AV flipped (P stationary, V moving):
        # pso[:, 128t:128t+65] accumulates [128 queries, 64 dims + denom].
        def emit_av(pso, gi, glen, jb, pview, pstep):
            st = 2 if pstep == 1024 else 1
            for q in range(glen):
                j = jb + q
                for t in range(4):
                    nc.tensor.matmul(
                        pso[:, 128 * t:128 * t + 65],
                        (pview[:, pstep * q + st * 128 * t:
                               pstep * q + st * 128 * t + st * 128:st]),
                        (vsb[:, 65 * j:65 * j + 65]),
                        start=(j == 0 and t == 0), stop=(j == 31),
                        skip_group_check=True)

        def emit_normalize(ic, pso):
            # normalize + pair-avg from [query, dim] psum (PSUM single-read
            # rule: copy dims to SBUF, then pair-add + scale on Pool)
            resb = resbs[ic % 2]
            for tq in range(4):
                rec = recs[(4 * ic + tq) % 2]
                vec.reciprocal(rec, pso[:, 128 * tq + 64:128 * tq + 65])
                osb = pav2s[(4 * ic + tq) % 2]
                vec.tensor_copy(osb, pso[:, 128 * tq:128 * tq + 64])
                pairs = osb.rearrange("p (e two) -> p e two", two=2)
                nc.gpsimd.tensor_add(resb[:, 32 * tq:32 * tq + 32],
                                     pairs[:, :, 0], pairs[:, :, 1])
                nc.gpsimd.tensor_scalar(resb[:, 32 * tq:32 * tq + 32],
                                        resb[:, 32 * tq:32 * tq + 32], rec,
                                        0.5, OP.mult, OP.mult)
            dma(out=out_a[512 * ic:512 * ic + 512, :].rearrange(
                    "(t p) e -> p t e", t=4),
                in_=resb.rearrange("p (t e) -> p t e", t=4))

        # flat software pipeline over all 8*16 groups: AV trails scores/exp
        # by 2 groups, normalize trails its last AV by 2 more groups.
        spools = [pa_, pb_, pc_]
        psos = {}
        pend = []       # (gc, ic, gi, glen, jb, pview, pstep)
        norm_pend = []  # (gc_when_av_emitted, ic)
        gc = 0          # pool-rotation beat (incl. conv beats)
        eb = 0          # exp beat (regular groups only) -> psb/fsc rotation

        def drain_one():
            _, pic, pgi, pglen, pjb, pv, pst = pend.pop(0)
            if pgi == 0:
                psos[pic] = po.tile([128, 512], FP, tag="o", name=f"o{pic}")
            emit_av(psos[pic], pgi, pglen, pjb, pv, pst)
            if pgi == len(GROUPS) - 1:
                norm_pend.append((gc, pic))

        # conv-group beats: interleave the 4 conv matmul chains into the
        # rotation at ic 3..6 starts (well after the permute-DMA roundtrip)
        cv_beats = {3: (0, 0), 4: (0, 1), 5: (1, 0), 6: (1, 1)}
        for ic in (() if ABL == "noattn" else range(8)):
            rhs_q = (ut[:, 3 * 512 * ic: 3 * 512 * ic + 1535: 3])
            for gi, glen in enumerate(GROUPS):
                while norm_pend and gc >= norm_pend[0][0] + 2:
                    _, pic0 = norm_pend.pop(0)
                    emit_normalize(pic0, psos[pic0])
                if gi == 0 and ic in cv_beats and ABL != "noconv":
                    oc, ch = cv_beats[ic]
                    cvt = spools[gc % 3].tile([128, 1024], FP, tag="s",
                                              name=f"cvt{oc}_{ch}")
                    emit_conv_group(oc, ch, cvt[:, 0:512])
                    gc += 1
                pool = spools[gc % 3]
                pss = pool.tile([128, 512 * glen], FP, tag="s",
                                name=f"s{ic}_{gi}")
                jb = 2 * gi
                for q in range(glen):
                    nc.tensor.matmul(
                        pss[:, 512 * q:512 * q + 512],
                        (ut[:, 3 * 128 * (jb + q) + 1:
                             3 * 128 * (jb + q) + 383: 3]),
                        rhs_q, start=True, stop=True, skip_group_check=True)
                if gi in ACT_GROUPS:
                    psb = psbs[eb % 3]
                    act.activation(psb[:, 0:512 * glen], pss[:, 0:512 * glen],
                                   AF.Exp, scale=0.125)
                    pend.append((gc, ic, gi, glen, jb, psb, 512))
                else:
                    fsc = fscs[eb % 3]
                    vec.tensor_scalar(fsc[:, 0:512 * glen],
                                      pss[:, 0:512 * glen],
                                      FE_A, FE_B, OP.mult, OP.add)
                    pend.append((gc, ic, gi, glen, jb, fsc.bitcast(BF), 1024))
                if len(pend) > 2:
                    drain_one()
                gc += 1
                eb += 1
        while pend:
            drain_one()
        for _, pic in norm_pend:
            emit_normalize(pic, psos[pic])


# =====================================================================
# Host side
# =====================================================================
_NC_CACHE = None


def _get_nc():
    global _NC_CACHE
    if _NC_CACHE is None:
        _NC_CACHE = build_device_program()
    return _NC_CACHE


def make_in_maps(x, qkv_w, bn_gamma, bn_beta, conv1_w, conv2_w):
    bfd = ml_dtypes.bfloat16
    x = np.asarray(x, np.float32)
    WT = np.ascontiguousarray(np.asarray(qkv_w, np.float32).T)   # [256, 1536]
    xT = np.ascontiguousarray(x.transpose(0, 2, 1))              # [2, 256, 4096]
    w1s = np.ascontiguousarray(
        0.5 * np.asarray(conv1_w, np.float32)[:, :, :, 0].transpose(1, 2, 0)
        .reshape(256, 768)).astype(bfd)                          # [i, dy*256+o]
    w2s = np.ascontiguousarray(
        0.5 * np.asarray(conv2_w, np.float32)[:, :, 0, :].transpose(1, 2, 0)
        .reshape(256, 768)).astype(bfd)
    gbar = np.ascontiguousarray(
        np.stack([np.asarray(bn_gamma, np.float32),
                  np.asarray(bn_beta, np.float32)], axis=1))     # [256, 2]
    idn = np.eye(128, dtype=np.float32)

    ilo = np.arange(64)
    in_maps = []
    for c in range(8):
        b, h = c // 4, c % 4
        head_cols = np.concatenate(
            [512 * g + 64 * h + np.arange(64) for g in range(8)])
        conv_cols = np.concatenate(
            [512 * g + 256 + np.arange(256) for g in range(8)])
        all_cols = np.concatenate([head_cols, conv_cols])
        mask = np.ones(4096, bool)
        mask[all_cols] = False
        miss_cols = np.nonzero(mask)[0]                          # 1536 cols
        xcb = np.ascontiguousarray(xT[b][:, all_cols]).astype(bfd)
        xrb = np.ascontiguousarray(np.concatenate(
            [xT[b][:, miss_cols], xT[1 - b]], axis=1)).astype(bfd)

        # slim conv-proj weights: col (j*8+rho)*128 + 64*mr + ilo
        #   -> WT col (3*rho+j)//8 + 3*(2h+mr) + 24*ilo   (j=2 scaled by 0.5)
        wcg = np.zeros((256, 3072), np.float32)
        for j in range(3):
            sc = 0.5 if j == 2 else 1.0
            for rho in range(8):
                o0 = (3 * rho + j) // 8
                for mr in range(2):
                    cols = o0 + 3 * (2 * h + mr) + 24 * ilo
                    wcg[:, (j * 8 + rho) * 128 + 64 * mr + ilo] = sc * WT[:, cols]
        # halo: lo (rho=7, ya=2h-1): o = 2 + 3*(2h-1) + 24*ilo   (h>=1)
        #       hi (rho=0, ya=2h+2): o = 3*(2h+2) + 24*ilo       (h<=2)
        wch = np.zeros((256, 128), np.float32)
        if h >= 1:
            wch[:, 0:64] = WT[:, 2 + 3 * (2 * h - 1) + 24 * ilo]
        if h <= 2:
            wch[:, 64:128] = WT[:, 3 * (2 * h + 2) + 24 * ilo]

        in_maps.append({
            "xcb": xcb, "xrb": xrb, "wq": WT.astype(bfd),
            "wcg": wcg.astype(bfd), "wch": wch.astype(bfd),
            "w1s": w1s, "w2s": w2s, "gb": gbar, "idn": idn,
        })
    return in_maps


def assemble(results):
    out = np.zeros((B, N, DIM), np.float32)
    for c in range(8):
        b, h = c // 4, c % 4
        out[b, :, 32 * h:32 * h + 32] = results[c]["out_a"]
        oc = results[c]["out_c"].reshape(256, 4, 128)
        out[b].reshape(256, 16, 256)[:, 4 * h:4 * h + 4, 128:256] = oc
    return out


def kernel(**inputs):
    nc = _get_nc()
    in_maps = make_in_maps(**inputs)
    res = bass_utils.run_bass_kernel_spmd(
        nc, in_maps, core_ids=list(range(8)),
        trace=bool(int(os.environ.get("KERNEL_TRACE", "0"))))
    out = assemble(res.results)
    if res.exec_time_ns is not None:
        print(f"HW exec time: {res.exec_time_ns} ns", file=sys.stderr)
        kernel.last_exec_time_ns = res.exec_time_ns
    kernel.last_results = res
    return out


kernel.last_exec_time_ns = None
kernel.last_results = None

